# revision 1
# baseline (speedup 1.0000x reference)
"""Sliding-window causal GQA attention with ALiBi for Trainium2, SPMD on 8
NeuronCores.

Problem (hardcoded): B=1, S=2048, D=2048, 16 query heads / 4 KV groups,
head_dim 128, window 512.

Sharding: tensor parallel over heads — core c owns KV group c//2 and query
head pair c%2 within that group (2 query heads per core, full sequence).
Wq/Wk/Wv are column-sharded by head, Wo row-sharded; each core produces a
full-shape partial of the output projection and the host sums the 8 partials
(replaces the all-reduce).

Device-side layout: the host passes x TRANSPOSED (xt = x.T, [D, S]). All
projections then emit transposed activations (qT/kT/vT = [head_dim, S]),
scores are computed as [keys, q] blocks — exactly the operand order the PE
array wants for the probs @ V matmul (keys on the contraction partition) —
and yT = [head_dim, q] is exactly the lhsT the output projection wants. The
only on-device transposes are 16 PE-transposes of V tiles.

Softmax: scores are small (|qk/sqrt(d)| ~ 4) and the ALiBi bias negative, so
fp32 exp never overflows and the max-subtraction pass is skipped. The
window/causal mask + ALiBi bias live in a host-precomputed [128, 640]
template added to the scores PSUM via an identity matmul on the PE. Row sums
come from a ones-vector matmul accumulated alongside the PV matmul; the
1/rowsum normalization is deferred to the output projection, where q sits on
the partition axis and a per-partition tensor_scalar multiply applies it —
so no PE instruction ever waits on the reciprocal chain (head-of-line
stalls on the in-order PE queue re-throttle the HAM clock gate).
"""

import math

import numpy as np
import ml_dtypes

import concourse.bass as bass
import concourse.mybir as mybir
import concourse.tile as tile
from concourse.masks import make_identity

BF16 = ml_dtypes.bfloat16

B, S, D = 1, 2048, 2048
NH, NKV, HD = 16, 4, 128
REP = NH // NKV          # query heads per KV group
WINDOW = 512
NCORES = 8
HPC = 2                  # query heads per core
QC = 512                 # q-chunk width (one PSUM bank of fp32)
NQC = S // QC            # 4
NKT = S // 128           # 16 key tiles
NDC = D // 128           # 16 contraction chunks
TW = WINDOW + 128        # 640: bias template width
NEG = -1.0e30

FP32 = mybir.dt.float32
BF = mybir.dt.bfloat16


def _alibi_slopes(n_heads: int) -> np.ndarray:
    def pow2_slopes(n):
        start = 2.0 ** (-(2.0 ** (-(math.log2(n) - 3))))
        return [start * start**i for i in range(n)]

    if math.log2(n_heads).is_integer():
        slopes = pow2_slopes(n_heads)
    else:
        closest = 2 ** math.floor(math.log2(n_heads))
        slopes = pow2_slopes(closest)
        slopes += pow2_slopes(2 * closest)[0::2][: n_heads - closest]
    return np.asarray(slopes, dtype=np.float32)


def _bias_templates() -> np.ndarray:
    """[NH, 128, TW] fp32. Template col c of key-tile row kc corresponds to
    query position q = k0 + c (k0 = key tile start). Valid iff kc <= c <=
    kc + WINDOW - 1; value -slope * (c - kc), else -1e30."""
    slopes = _alibi_slopes(NH)
    kc = np.arange(128)[:, None]
    c = np.arange(TW)[None, :]
    dist = (c - kc).astype(np.float32)
    valid = (dist >= 0) & (dist <= WINDOW - 1)
    out = np.empty((NH, 128, TW), np.float32)
    for h in range(NH):
        out[h] = np.where(valid, -slopes[h] * dist, NEG)
    return out


def _split_waits(nc, maxw=1):
    """This container's walrus rejects instructions with more than one sync
    wait command; hoist extra waits onto preceding same-engine NoOps."""
    plan = {}
    si_type = None
    for bb in nc.main_func.blocks:
        for ins in bb.instructions:
            si = ins.sync_info
            waits = list(si.on_wait) if si and si.on_wait else []
            if len(waits) > maxw:
                si_type = type(si)
                extra = [waits[i:i + maxw] for i in range(0, len(waits) - maxw, maxw)]
                keep = waits[len(extra) * maxw:]
                plan[ins.name] = (extra, keep)
    if not plan:
        return 0
    nops = {}
    nop_names = set()
    for name, (extra, _keep) in plan.items():
        target = nc.inst_map[name]
        eng = nc.engines[target.engine]
        lst = []
        for chunk in extra:
            nop = eng.nop(nofuse=True).ins
            nop.sync_info = si_type(on_wait=chunk, on_update=[])
            lst.append(nop)
            nop_names.add(nop.name)
        nops[name] = lst
    for bb in nc.main_func.blocks:
        insts = list(bb.instructions)
        out = []
        changed = False
        for ins in insts:
            if ins.name in nop_names:
                changed = True
                continue
            if ins.name in plan:
                _extra, keep = plan[ins.name]
                si = ins.sync_info
                upd = list(si.on_update) if si and si.on_update else []
                ins.sync_info = si_type(on_wait=keep, on_update=upd)
                out.extend(nops[ins.name])
                changed = True
            out.append(ins)
        if changed:
            bb.instructions = out
    return len(plan)


def _kt_range(qc):
    """Key tiles feeding q-chunk qc: keys [qc*QC - WINDOW + 1, qc*QC + QC - 1]."""
    lo = max(0, (qc * QC - WINDOW + 1) // 128)
    hi = (qc * QC + QC - 1) // 128
    return lo, hi


def _build_program():
    nc = bass.Bass()

    # weight/bias inputs arrive pre-shuffled to partition-major layouts so
    # every input DMA is fully contiguous per partition row
    xt = nc.dram_tensor("xt", [D, S], BF, kind="ExternalInput")
    wq = nc.dram_tensor("wq", [128, NDC * HPC * HD], BF, kind="ExternalInput")
    wk = nc.dram_tensor("wk", [128, NDC * HD], BF, kind="ExternalInput")
    wv = nc.dram_tensor("wv", [128, NDC * HD], BF, kind="ExternalInput")
    wo = nc.dram_tensor("wo", [128, HPC * D], BF, kind="ExternalInput")
    biast = nc.dram_tensor("biast", [128, HPC * TW], BF, kind="ExternalInput")
    out = nc.dram_tensor("out", [S, D], mybir.dt.float16, kind="ExternalOutput")

    with tile.TileContext(nc) as tc:
        with tc.tile_pool(name="persist", bufs=1) as persist:
            xt_sb = [persist.tile([128, S], BF, name=f"xt{d}") for d in range(NDC)]
            wq_sb = [persist.tile([128, HPC * HD], BF, name=f"wq{d}") for d in range(NDC)]
            wk_sb = [persist.tile([128, HD], BF, name=f"wk{d}") for d in range(NDC)]
            wv_sb = [persist.tile([128, HD], BF, name=f"wv{d}") for d in range(NDC)]
            wo_sb = persist.tile([128, HPC, D], BF)
            bias_sb = persist.tile([128, HPC, TW], BF)
            qt_sb = [persist.tile([128, S], BF, name=f"qt{h}") for h in range(HPC)]
            kt_sb = [persist.tile([128, QC], BF, name=f"kt{i}") for i in range(NQC)]
            vt_sb = persist.tile([128, S], BF)
            v_sb = [persist.tile([128, HD], BF, name=f"v{i}") for i in range(NKT)]
            # normalized y^T per (h, qc)
            yt_sb = [
                [persist.tile([128, QC], BF, name=f"yt{h}_{q}") for q in range(NQC)]
                for h in range(HPC)
            ]
            rc_sb = [
                [persist.tile([128, 4], FP32, name=f"rc{h}_{q}") for q in range(NQC)]
                for h in range(HPC)
            ]
            ident = persist.tile([128, 128], BF)
            ones_k = persist.tile([128, 1], BF)

            def dma_split(dst2d, src2d, pieces, eng=None):
                # one DMA descriptor drives a single ~21GB/s engine and costs
                # ~610ns of serial issue on its sequencer; split large loads
                # across queues and spread issue over both HWDGE sequencers
                eng = eng or nc.sync
                n = dst2d.shape[-1]
                step = n // pieces
                for i in range(pieces):
                    eng.dma_start(
                        out=dst2d[:, i * step:(i + 1) * step],
                        in_=src2d[:, i * step:(i + 1) * step],
                    )

            # priority order: k-proj inputs first, then q, then the rest.
            # weights land as one small DMA per contraction chunk so both
            # queue parallelism and dependency granularity are per-chunk
            for dch in range(NDC):
                nc.sync.dma_start(out=wk_sb[dch], in_=wk[:, dch * HD:(dch + 1) * HD])
            for dch in range(NDC):
                nc.scalar.dma_start(
                    out=wq_sb[dch],
                    in_=wq[:, dch * HPC * HD:(dch + 1) * HPC * HD],
                )
            for dch in range(NDC):
                eng = nc.sync if dch % 2 == 0 else nc.scalar
                dma_split(xt_sb[dch], xt[dch * 128:(dch + 1) * 128, :], 2, eng=eng)
            for dch in range(NDC):
                nc.sync.dma_start(out=wv_sb[dch], in_=wv[:, dch * HD:(dch + 1) * HD])
            dma_split(wo_sb.rearrange("p h n -> p (h n)"), wo[:, :], 8, eng=nc.scalar)
            dma_split(bias_sb.rearrange("p h c -> p (h c)"), biast[:, :], 2, eng=nc.scalar)
            make_identity(nc, ident)
            nc.vector.memset(ones_k, 1.0)

            # ---- phase 1: projections (all emitted transposed) ----
            with tc.tile_pool(name="proj_ps", bufs=4, space="PSUM") as proj_ps, \
                 tc.tile_pool(name="tp_ps", bufs=2, space="PSUM") as tp_ps:

                def proj_T(w_sb, w_col0, dsts):
                    pss = []
                    for sc in range(NQC):
                        ps = proj_ps.tile([128, QC], FP32, tag="proj")
                        pss.append(ps)
                    for dch in range(NDC):
                        for sc in range(NQC):
                            nc.tensor.matmul(
                                pss[sc],
                                w_sb[dch][:, w_col0:w_col0 + 128],
                                xt_sb[dch][:, sc * QC:(sc + 1) * QC],
                                start=(dch == 0),
                                stop=(dch == NDC - 1),
                            )
                    for sc in range(NQC):
                        nc.scalar.copy(out=dsts[sc], in_=pss[sc])

                # order: k first (attention critical path), then q0, v, q1
                proj_T(wk_sb, 0, kt_sb)
                proj_T(wq_sb, 0, [qt_sb[0][:, sc * QC:(sc + 1) * QC] for sc in range(NQC)])
                proj_T(wv_sb, 0, [vt_sb[:, sc * QC:(sc + 1) * QC] for sc in range(NQC)])
                for kt in range(NKT):
                    tp = tp_ps.tile([128, 128], BF, tag="tp")
                    nc.tensor.transpose(tp, vt_sb[:, kt * 128:(kt + 1) * 128], ident)
                    nc.scalar.copy(out=v_sb[kt], in_=tp)
                proj_T(wq_sb, HD, [qt_sb[1][:, sc * QC:(sc + 1) * QC] for sc in range(NQC)])

            # ---- phase 2: attention + output projection, per q-chunk ----
            with tc.tile_pool(name="sc_ps", bufs=2, space="PSUM") as sc_ps, \
                 tc.tile_pool(name="yt_ps", bufs=2, space="PSUM") as yt_ps, \
                 tc.tile_pool(name="rs_ps", bufs=2, space="PSUM") as rs_ps, \
                 tc.tile_pool(name="op_ps", bufs=2, space="PSUM") as op_ps, \
                 tc.tile_pool(name="et_sb", bufs=4) as et_pool, \
                 tc.tile_pool(name="small_sb", bufs=4) as small, \
                 tc.tile_pool(name="op_sb", bufs=4) as op_sb, \
                 tc.tile_pool(name="r_dram", bufs=4, space="DRAM") as r_dram:

                def attention(h, qc):
                    q0 = qc * QC
                    klo, khi = _kt_range(qc)
                    y_ps = yt_ps.tile([128, QC], FP32, tag="y")
                    r_ps = rs_ps.tile([1, QC], FP32, tag="r")
                    # shifted-window PSUM accumulation: the first matmul
                    # (start=True) must cover all 512 columns since
                    # has_written is per-element; key tile 4*qc always does.
                    kts = [4 * qc] + [t for t in range(klo, khi + 1) if t != 4 * qc]
                    for i, kt in enumerate(kts):
                        k0 = kt * 128
                        q_lo = max(q0, k0)
                        q_hi = min(q0 + QC - 1, k0 + TW - 1)
                        w = q_hi - q_lo + 1
                        first, last = i == 0, i == len(kts) - 1
                        s_ps = sc_ps.tile([128, QC], FP32, tag="sc")
                        nc.tensor.matmul(
                            s_ps[:, :w],
                            kt_sb[kt // 4][:, (kt % 4) * 128:(kt % 4) * 128 + 128],
                            qt_sb[h][:, q_lo:q_hi + 1],
                            start=True,
                            stop=False,
                        )
                        nc.tensor.matmul(
                            s_ps[:, :w],
                            ident,
                            bias_sb[:, h, q_lo - k0:q_lo - k0 + w],
                            start=False,
                            stop=True,
                        )
                        et = et_pool.tile([128, QC], BF, tag="et")
                        nc.scalar.activation(
                            out=et[:, :w],
                            in_=s_ps[:, :w],
                            func=mybir.ActivationFunctionType.Exp,
                        )
                        nc.tensor.matmul(
                            y_ps[:, q_lo - q0:q_lo - q0 + w],
                            v_sb[kt],
                            et[:, :w],
                            start=first,
                            stop=last,
                            skip_group_check=True,
                        )
                        nc.tensor.matmul(
                            r_ps[:, q_lo - q0:q_lo - q0 + w],
                            ones_k,
                            et[:, :w],
                            start=first,
                            stop=last,
                            skip_group_check=True,
                        )
                    # rowsum normalization, entirely off the PE stream:
                    # reciprocal of the [1,512] sums, bounce through DRAM,
                    # stride-0 partition broadcast (128 contiguous 2KB
                    # reads), one DVE multiply pre-scales y^T
                    yun = small.tile([128, QC], FP32, tag="yun")
                    nc.scalar.copy(out=yun, in_=y_ps)
                    recip_row = small.tile([1, QC], FP32, tag="rr")
                    nc.vector.reciprocal(recip_row, r_ps)
                    rd1 = r_dram.tile([1, QC], FP32, tag="rd1")
                    nc.sync.dma_start(out=rd1, in_=recip_row)
                    rb = small.tile([128, QC], FP32, tag="rb")
                    bc = bass.AP(tensor=rd1.tensor, offset=rd1.offset,
                                 ap=[[0, 128], [1, QC]])
                    nc.sync.dma_start(out=rb, in_=bc)
                    nc.gpsimd.tensor_tensor(
                        yt_sb[h][qc], yun, rb, mybir.AluOpType.mult
                    )

                def outproj(qc):
                    for sti in range(4):
                        st = qc * 4 + sti
                        for ncol in range(D // QC):
                            ps = op_ps.tile([128, QC], FP32, tag="op")
                            for h in range(HPC):
                                nc.tensor.matmul(
                                    ps,
                                    yt_sb[h][qc][:, sti * 128:(sti + 1) * 128],
                                    wo_sb[:, h, ncol * QC:(ncol + 1) * QC],
                                    start=(h == 0),
                                    stop=(h == HPC - 1),
                                )
                            stg = op_sb.tile([128, QC], mybir.dt.float16, tag="stg")
                            if ncol % 2 == 0:
                                nc.scalar.copy(out=stg, in_=ps)
                            else:
                                nc.vector.tensor_copy(stg, ps)
                            nc.sync.dma_start(
                                out=out[st * 128:(st + 1) * 128,
                                        ncol * QC:(ncol + 1) * QC],
                                in_=stg,
                            )

                # out-proj lags attention by one q-chunk so the normalize
                # chain's latency never backs up the PE stream
                for qc in range(NQC):
                    for h in range(HPC):
                        attention(h, qc)
                    if qc > 0:
                        outproj(qc - 1)
                outproj(NQC - 1)

    _split_waits(nc, maxw=1)
    return nc


_NC_CACHE = None


def _get_program():
    global _NC_CACHE
    if _NC_CACHE is None:
        _NC_CACHE = _build_program()
    return _NC_CACHE


def _shuffle_chunks(w, cols):
    """[D, cols] -> [128, NDC*cols] partition-major contiguous layout."""
    return np.ascontiguousarray(
        w.reshape(NDC, 128, cols).transpose(1, 0, 2).reshape(128, NDC * cols)
    )


def build_in_maps(x, Wq, Wk, Wv, Wo):
    x = np.asarray(x, np.float32)
    Wq = np.asarray(Wq, np.float32)
    Wk = np.asarray(Wk, np.float32)
    Wv = np.asarray(Wv, np.float32)
    Wo = np.asarray(Wo, np.float32)

    xt = np.ascontiguousarray(x[0].T).astype(BF16)
    wq_s = (Wq * (1.0 / math.sqrt(HD))).astype(BF16)
    wk_s = Wk.astype(BF16)
    wv_s = Wv.astype(BF16)
    wo_s = Wo.astype(BF16)
    templates = _bias_templates()

    in_maps = []
    for c in range(NCORES):
        g, hp = c // HPC, c % HPC
        heads = [g * REP + hp * HPC + r for r in range(HPC)]
        wo_rows = wo_s[heads[0] * HD:(heads[-1] + 1) * HD, :]  # [256, D]
        in_maps.append(
            {
                "xt": xt,
                "wq": _shuffle_chunks(
                    wq_s[:, heads[0] * HD:(heads[-1] + 1) * HD], HPC * HD
                ),
                "wk": _shuffle_chunks(wk_s[:, g * HD:(g + 1) * HD], HD),
                "wv": _shuffle_chunks(wv_s[:, g * HD:(g + 1) * HD], HD),
                "wo": np.ascontiguousarray(
                    wo_rows.reshape(HPC, 128, D).transpose(1, 0, 2).reshape(128, HPC * D)
                ),
                "biast": np.ascontiguousarray(
                    templates[heads].transpose(1, 0, 2).reshape(128, HPC * TW)
                ).astype(BF16),
            }
        )
    return in_maps


_last_in_maps = None


def kernel(x, Wq, Wk, Wv, Wo):
    from concourse.bass_utils import run_bass_kernel_spmd

    global _last_in_maps
    in_maps = build_in_maps(x, Wq, Wk, Wv, Wo)
    _last_in_maps = in_maps

    nc = _get_program()
    res = run_bass_kernel_spmd(nc, in_maps, list(range(NCORES)))
    acc = res.results[0]["out"].astype(np.float64)
    for c in range(1, NCORES):
        acc += res.results[c]["out"]
    return acc.astype(np.float32).reshape(B, S, D)



# revision 5
# speedup vs baseline: 1.3984x; 1.3984x over previous
"""Sliding-window causal GQA attention with ALiBi for Trainium2, SPMD on 8
NeuronCores.

Problem (hardcoded): B=1, S=2048, D=2048, 16 query heads / 4 KV groups,
head_dim 128, window 512.

Sharding: tensor parallel over heads — core c owns KV group c//2 and query
head pair c%2 within that group (2 query heads per core, full sequence).
Wq/Wk/Wv are column-sharded by head, Wo row-sharded; each core produces a
full-shape partial of the output projection and the host sums the 8 partials
(replaces the all-reduce).

Device-side layout: the host passes x TRANSPOSED (xt = x.T, [D, S]). All
projections then emit transposed activations (qT/kT/vT = [head_dim, S]),
scores are computed as [keys, q] blocks — exactly the operand order the PE
array wants for the probs @ V matmul (keys on the contraction partition) —
and yT = [head_dim, q] is exactly the lhsT the output projection wants. The
V tiles are turned into [key, head_dim] layout by 16 SBUF->SBUF xbar DMA
transposes (no PE/DVE time).

v2 structure (driven by the v1 profile: scalar queue 112us busy, sync queue
91us busy issuing 170 small DMAs, PE only 65% occupied and HAM-throttled):
 - every weight arrives as ONE large DMA into one contiguous SBUF tile; x
   arrives as 16 per-chunk DMAs alternating between the two HWDGE queues so
   the first projection can start after ~2 chunks.
 - phase 1a interleaves the K and Q0 projections per contraction chunk
   (8 PSUM banks) so PE consumption (~1.7us/chunk) outpaces DMA arrival
   (~1.4us/chunk) and the PE never goes cold waiting on x.
 - phase 1b does V and Q1 the same way from resident x.
 - the softmax row-sum is accumulated with an all-ones [128,128] stationary
   operand so the PSUM result is replicated across all 128 partitions;
   1/rowsum is then a parallel DVE reciprocal_approx_fast and the normalize
   is a single DVE tensor_tensor multiply reading y straight out of PSUM —
   no DRAM bounce, no single-partition 3.3us reciprocal, no gpsimd.
 - output staging copies alternate scalar/DVE and feed 16 large [128,2048]
   store DMAs.

Softmax: scores are small (|qk/sqrt(d)| ~ 4) and the ALiBi bias negative, so
fp32 exp never overflows and the max-subtraction pass is skipped. The
window/causal mask + ALiBi bias live in a host-precomputed [128, 640]
template added to the scores PSUM via an identity matmul on the PE.
"""

import math

import numpy as np
import ml_dtypes

import concourse.bass as bass
import concourse.mybir as mybir
import concourse.tile as tile
from concourse.masks import make_identity

BF16 = ml_dtypes.bfloat16

B, S, D = 1, 2048, 2048
NH, NKV, HD = 16, 4, 128
REP = NH // NKV          # query heads per KV group
WINDOW = 512
NCORES = 8
HPC = 2                  # query heads per core
QC = 512                 # q-chunk width (one PSUM bank of fp32)
NQC = S // QC            # 4
NKT = S // 128           # 16 key tiles
NDC = D // 128           # 16 contraction chunks
TW = WINDOW + 128        # 640: bias template width
NEG = -1.0e30

FP32 = mybir.dt.float32
BF = mybir.dt.bfloat16


def _alibi_slopes(n_heads: int) -> np.ndarray:
    def pow2_slopes(n):
        start = 2.0 ** (-(2.0 ** (-(math.log2(n) - 3))))
        return [start * start**i for i in range(n)]

    if math.log2(n_heads).is_integer():
        slopes = pow2_slopes(n_heads)
    else:
        closest = 2 ** math.floor(math.log2(n_heads))
        slopes = pow2_slopes(closest)
        slopes += pow2_slopes(2 * closest)[0::2][: n_heads - closest]
    return np.asarray(slopes, dtype=np.float32)


def _bias_templates() -> np.ndarray:
    """[NH, 128, TW] fp32. Template col c of key-tile row kc corresponds to
    query position q = k0 + c (k0 = key tile start). Valid iff kc <= c <=
    kc + WINDOW - 1; value -slope * (c - kc), else -1e30."""
    slopes = _alibi_slopes(NH)
    kc = np.arange(128)[:, None]
    c = np.arange(TW)[None, :]
    dist = (c - kc).astype(np.float32)
    valid = (dist >= 0) & (dist <= WINDOW - 1)
    out = np.empty((NH, 128, TW), np.float32)
    for h in range(NH):
        out[h] = np.where(valid, -slopes[h] * dist, NEG)
    return out


def _split_waits(nc, maxw=1):
    """This container's walrus rejects instructions with more than one sync
    wait command; hoist extra waits onto preceding same-engine NoOps."""
    plan = {}
    si_type = None
    for bb in nc.main_func.blocks:
        for ins in bb.instructions:
            si = ins.sync_info
            waits = list(si.on_wait) if si and si.on_wait else []
            if len(waits) > maxw:
                si_type = type(si)
                extra = [waits[i:i + maxw] for i in range(0, len(waits) - maxw, maxw)]
                keep = waits[len(extra) * maxw:]
                plan[ins.name] = (extra, keep)
    if not plan:
        return 0
    nops = {}
    nop_names = set()
    for name, (extra, _keep) in plan.items():
        target = nc.inst_map[name]
        eng = nc.engines[target.engine]
        lst = []
        for chunk in extra:
            nop = eng.nop(nofuse=True).ins
            nop.sync_info = si_type(on_wait=chunk, on_update=[])
            lst.append(nop)
            nop_names.add(nop.name)
        nops[name] = lst
    for bb in nc.main_func.blocks:
        insts = list(bb.instructions)
        out = []
        changed = False
        for ins in insts:
            if ins.name in nop_names:
                changed = True
                continue
            if ins.name in plan:
                _extra, keep = plan[ins.name]
                si = ins.sync_info
                upd = list(si.on_update) if si and si.on_update else []
                ins.sync_info = si_type(on_wait=keep, on_update=upd)
                out.extend(nops[ins.name])
                changed = True
            out.append(ins)
        if changed:
            bb.instructions = out
    return len(plan)


def _kt_range(qc):
    """Key tiles feeding q-chunk qc: keys [qc*QC - WINDOW + 1, qc*QC + QC - 1]."""
    lo = max(0, (qc * QC - WINDOW + 1) // 128)
    hi = (qc * QC + QC - 1) // 128
    return lo, hi


def _build_program():
    nc = bass.Bass()

    # weight/bias inputs arrive pre-shuffled to partition-major layouts so
    # every input DMA is fully contiguous per partition row
    xt = nc.dram_tensor("xt", [D, S], BF, kind="ExternalInput")
    wq = nc.dram_tensor("wq", [128, NDC * HPC * HD], BF, kind="ExternalInput")
    wk = nc.dram_tensor("wk", [128, NDC * HD], BF, kind="ExternalInput")
    wv = nc.dram_tensor("wv", [128, NDC * HD], BF, kind="ExternalInput")
    wo = nc.dram_tensor("wo", [128, HPC * D], BF, kind="ExternalInput")
    biast = nc.dram_tensor("biast", [128, HPC * TW], BF, kind="ExternalInput")
    out = nc.dram_tensor("out", [S, D], mybir.dt.float16, kind="ExternalOutput")

    with tile.TileContext(nc) as tc:
        with tc.tile_pool(name="persist", bufs=1) as persist:
            xt_sb = [persist.tile([128, S], BF, name=f"xt{d}") for d in range(NDC)]
            wq_sb = persist.tile([128, NDC * HPC * HD], BF)
            wk_sb = persist.tile([128, NDC * HD], BF)
            wv_sb = persist.tile([128, NDC * HD], BF)
            wo_sb = persist.tile([128, HPC, D], BF)
            bias_sb = persist.tile([128, HPC, TW], BF)
            qt_sb = [persist.tile([128, S], BF, name=f"qt{h}") for h in range(HPC)]
            kt_sb = persist.tile([128, S], BF)
            vt_sb = persist.tile([128, S], BF)
            v_sb = persist.tile([128, NKT, HD], BF)
            # normalized y^T per (h, qc): [hd, q]
            yt_sb = persist.tile([128, HPC, S], BF)
            ident = persist.tile([128, 128], BF)
            ones_sq = persist.tile([128, 128], BF)

            # input DMAs: one large descriptor per weight, x per-chunk on
            # alternating HWDGE queues (completion order = program order per
            # queue, so chunk 0 lands first and the K/Q0 loop starts early)
            nc.sync.dma_start(out=wk_sb, in_=wk[:, :])
            nc.scalar.dma_start(out=wq_sb, in_=wq[:, :])
            for dch in range(NDC):
                eng = nc.sync if dch % 2 == 0 else nc.scalar
                eng.dma_start(out=xt_sb[dch], in_=xt[dch * 128:(dch + 1) * 128, :])
            nc.sync.dma_start(out=wv_sb, in_=wv[:, :])
            nc.scalar.dma_start(out=wo_sb.rearrange("p h n -> p (h n)"), in_=wo[:, :])
            nc.scalar.dma_start(out=bias_sb.rearrange("p h c -> p (h c)"), in_=biast[:, :])
            make_identity(nc, ident)
            nc.vector.memset(ones_sq, 1.0)

            # ---- phase 1: projections (all emitted transposed) ----
            # Two interleaved projections at a time = 8 PSUM banks; PE burns
            # 4096 cols (~1.7us) per x chunk, above the ~1.4us DMA arrival
            # rate, so phase 1a is DMA-overlapped and 1b runs from SBUF.
            with tc.tile_pool(name="proj_ps", bufs=8, space="PSUM") as proj_ps:

                def proj_pair(specs):
                    # specs: list of (lhsT getter, dst getter) run interleaved
                    pss = [
                        [
                            proj_ps.tile(
                                [128, QC], FP32, tag="proj", name=f"pp{si}_{sc}"
                            )
                            for sc in range(NQC)
                        ]
                        for si in range(len(specs))
                    ]
                    for dch in range(NDC):
                        for si, (lhs_fn, _dst) in enumerate(specs):
                            lhsT = lhs_fn(dch)
                            for sc in range(NQC):
                                nc.tensor.matmul(
                                    pss[si][sc],
                                    lhsT,
                                    xt_sb[dch][:, sc * QC:(sc + 1) * QC],
                                    start=(dch == 0),
                                    stop=(dch == NDC - 1),
                                    skip_group_check=True,
                                )
                    for si, (_lhs, dst_fn) in enumerate(specs):
                        for sc in range(NQC):
                            if (si * NQC + sc) % 2 == 0:
                                nc.vector.tensor_copy(dst_fn(sc), pss[si][sc])
                            else:
                                nc.scalar.copy(out=dst_fn(sc), in_=pss[si][sc])

                # 1a: K + Q0 (attention-critical operands first)
                proj_pair([
                    (lambda d: wk_sb[:, d * HD:(d + 1) * HD],
                     lambda sc: kt_sb[:, sc * QC:(sc + 1) * QC]),
                    (lambda d: wq_sb[:, d * HPC * HD:d * HPC * HD + HD],
                     lambda sc: qt_sb[0][:, sc * QC:(sc + 1) * QC]),
                ])
                # 1b: V + Q1
                proj_pair([
                    (lambda d: wv_sb[:, d * HD:(d + 1) * HD],
                     lambda sc: vt_sb[:, sc * QC:(sc + 1) * QC]),
                    (lambda d: wq_sb[:, d * HPC * HD + HD:(d + 1) * HPC * HD],
                     lambda sc: qt_sb[1][:, sc * QC:(sc + 1) * QC]),
                ])
                # V tiles -> [key, head_dim] via xbar DMA transpose (frees
                # PE/scalar entirely; 16 SBUF->SBUF transposes on sync queue)
                for kt in range(NKT):
                    nc.sync.dma_start_transpose(
                        v_sb[:, kt, :], vt_sb[:, kt * 128:(kt + 1) * 128]
                    )

            # ---- phase 2: attention + output projection, per q-chunk ----
            with tc.tile_pool(name="sc_ps", bufs=2, space="PSUM") as sc_ps, \
                 tc.tile_pool(name="yt_ps", bufs=2, space="PSUM") as yt_ps, \
                 tc.tile_pool(name="rs_ps", bufs=2, space="PSUM") as rs_ps, \
                 tc.tile_pool(name="op_ps", bufs=2, space="PSUM") as op_ps, \
                 tc.tile_pool(name="et_sb", bufs=4) as et_pool, \
                 tc.tile_pool(name="rc_sb", bufs=2) as rc_pool, \
                 tc.tile_pool(name="stage_sb", bufs=2) as stage_pool:

                def attention(h, qc):
                    q0 = qc * QC
                    klo, khi = _kt_range(qc)
                    y_ps = yt_ps.tile([128, QC], FP32, tag="y")
                    r_ps = rs_ps.tile([128, QC], FP32, tag="r")
                    # shifted-window PSUM accumulation: the first matmul
                    # (start=True) must cover all 512 columns since
                    # has_written is per-element; key tile 4*qc always does.
                    kts = [4 * qc] + [t for t in range(klo, khi + 1) if t != 4 * qc]
                    for i, kt in enumerate(kts):
                        k0 = kt * 128
                        q_lo = max(q0, k0)
                        q_hi = min(q0 + QC - 1, k0 + TW - 1)
                        w = q_hi - q_lo + 1
                        first, last = i == 0, i == len(kts) - 1
                        s_ps = sc_ps.tile([128, QC], FP32, tag="sc")
                        nc.tensor.matmul(
                            s_ps[:, :w],
                            kt_sb[:, kt * 128:kt * 128 + 128],
                            qt_sb[h][:, q_lo:q_hi + 1],
                            start=True,
                            stop=False,
                        )
                        nc.tensor.matmul(
                            s_ps[:, :w],
                            ident,
                            bias_sb[:, h, q_lo - k0:q_lo - k0 + w],
                            start=False,
                            stop=True,
                        )
                        et = et_pool.tile([128, QC], BF, tag="et")
                        nc.scalar.activation(
                            out=et[:, :w],
                            in_=s_ps[:, :w],
                            func=mybir.ActivationFunctionType.Exp,
                        )
                        nc.tensor.matmul(
                            y_ps[:, q_lo - q0:q_lo - q0 + w],
                            v_sb[:, kt, :],
                            et[:, :w],
                            start=first,
                            stop=last,
                            skip_group_check=True,
                        )
                        # all-ones stationary operand -> row-sum replicated
                        # across all 128 PSUM partitions (feeds a parallel
                        # DVE reciprocal + broadcast-free normalize)
                        nc.tensor.matmul(
                            r_ps[:, q_lo - q0:q_lo - q0 + w],
                            ones_sq,
                            et[:, :w],
                            start=first,
                            stop=last,
                            skip_group_check=True,
                        )
                    # 1/rowsum in 4 plain DVE ops (the fused custom-DVE
                    # reciprocal doesn't compile on this walrus): magic-number
                    # seed (max rel err 3.4%) + one Newton-Raphson step
                    # (-> 1.2e-3 max), sign carried through the last two fused
                    # scalar_tensor_tensor ops. rowsums are positive normals
                    # so the bit trick is safe.
                    y0 = rc_pool.tile([128, QC], mybir.dt.int32, tag="y0")
                    nc.vector.tensor_scalar(
                        out=y0,
                        in0=r_ps.bitcast(mybir.dt.int32),
                        scalar1=-1,
                        scalar2=0x7EF311C3,
                        op0=mybir.AluOpType.mult,
                        op1=mybir.AluOpType.add,
                    )
                    y0f = y0.bitcast(FP32)
                    t = rc_pool.tile([128, QC], FP32, tag="t")
                    nc.vector.tensor_tensor(t, r_ps, y0f, mybir.AluOpType.mult)
                    negy1 = rc_pool.tile([128, QC], FP32, tag="ny")
                    nc.vector.scalar_tensor_tensor(
                        negy1, t, 2.0, y0f,
                        mybir.AluOpType.subtract, mybir.AluOpType.mult,
                    )
                    nc.vector.scalar_tensor_tensor(
                        yt_sb[:, h, q0:q0 + QC], y_ps, -1.0, negy1,
                        mybir.AluOpType.mult, mybir.AluOpType.mult,
                    )

                def outproj(qc):
                    for sti in range(4):
                        st = qc * 4 + sti
                        stage = stage_pool.tile([128, D], mybir.dt.float16, tag="stg")
                        for ncol in range(D // QC):
                            ps = op_ps.tile([128, QC], FP32, tag="op")
                            for h in range(HPC):
                                nc.tensor.matmul(
                                    ps,
                                    yt_sb[:, h, st * 128:(st + 1) * 128],
                                    wo_sb[:, h, ncol * QC:(ncol + 1) * QC],
                                    start=(h == 0),
                                    stop=(h == HPC - 1),
                                )
                            if ncol % 2 == 0:
                                nc.scalar.copy(
                                    out=stage[:, ncol * QC:(ncol + 1) * QC], in_=ps
                                )
                            else:
                                nc.vector.tensor_copy(
                                    stage[:, ncol * QC:(ncol + 1) * QC], ps
                                )
                        nc.sync.dma_start(
                            out=out[st * 128:(st + 1) * 128, :], in_=stage
                        )

                # out-proj lags attention by one q-chunk so the normalize
                # chain's latency never backs up the PE stream
                for qc in range(NQC):
                    for h in range(HPC):
                        attention(h, qc)
                    if qc > 0:
                        outproj(qc - 1)
                outproj(NQC - 1)

    _split_waits(nc, maxw=1)
    return nc


_NC_CACHE = None


def _get_program():
    global _NC_CACHE
    if _NC_CACHE is None:
        _NC_CACHE = _build_program()
    return _NC_CACHE


def _shuffle_chunks(w, cols):
    """[D, cols] -> [128, NDC*cols] partition-major contiguous layout."""
    return np.ascontiguousarray(
        w.reshape(NDC, 128, cols).transpose(1, 0, 2).reshape(128, NDC * cols)
    )


def build_in_maps(x, Wq, Wk, Wv, Wo):
    x = np.asarray(x, np.float32)
    Wq = np.asarray(Wq, np.float32)
    Wk = np.asarray(Wk, np.float32)
    Wv = np.asarray(Wv, np.float32)
    Wo = np.asarray(Wo, np.float32)

    xt = np.ascontiguousarray(x[0].T).astype(BF16)
    wq_s = (Wq * (1.0 / math.sqrt(HD))).astype(BF16)
    wk_s = Wk.astype(BF16)
    wv_s = Wv.astype(BF16)
    wo_s = Wo.astype(BF16)
    templates = _bias_templates()

    in_maps = []
    for c in range(NCORES):
        g, hp = c // HPC, c % HPC
        heads = [g * REP + hp * HPC + r for r in range(HPC)]
        wo_rows = wo_s[heads[0] * HD:(heads[-1] + 1) * HD, :]  # [256, D]
        in_maps.append(
            {
                "xt": xt,
                "wq": _shuffle_chunks(
                    wq_s[:, heads[0] * HD:(heads[-1] + 1) * HD], HPC * HD
                ),
                "wk": _shuffle_chunks(wk_s[:, g * HD:(g + 1) * HD], HD),
                "wv": _shuffle_chunks(wv_s[:, g * HD:(g + 1) * HD], HD),
                "wo": np.ascontiguousarray(
                    wo_rows.reshape(HPC, 128, D).transpose(1, 0, 2).reshape(128, HPC * D)
                ),
                "biast": np.ascontiguousarray(
                    templates[heads].transpose(1, 0, 2).reshape(128, HPC * TW)
                ).astype(BF16),
            }
        )
    return in_maps


_last_in_maps = None


def kernel(x, Wq, Wk, Wv, Wo):
    from concourse.bass_utils import run_bass_kernel_spmd

    global _last_in_maps
    in_maps = build_in_maps(x, Wq, Wk, Wv, Wo)
    _last_in_maps = in_maps

    nc = _get_program()
    res = run_bass_kernel_spmd(nc, in_maps, list(range(NCORES)))
    acc = res.results[0]["out"].astype(np.float64)
    for c in range(1, NCORES):
        acc += res.results[c]["out"]
    return acc.astype(np.float32).reshape(B, S, D)
